# revision 3
# baseline (speedup 1.0000x reference)
"""Trainium2 Bass kernel for dense_cnn problem.

Math (per batch element n, C=128 channels, H=W=56, G=8):
  t1 = conv_h(x, w1)          5-tap conv over H with full channel mixing
  t3 = dwconv_h(t1, w3)       3-tap depthwise conv over H
  t4[g] = sum_{c,k} x[c, h, w+2k-2] * w4[c,k,g]   (3 width taps, dil 2)
  out[c] = t3[c] * t4[c % 8]

Device strategy (data-parallel, 4 batch elems per core across 8 cores):
  - PE does only the channel-mixing work: 5 matmul taps for t1 and
    3 width taps for t4 (w4 broadcast to 128 output channels on the
    host so the final combine is elementwise).  That is 8 taps of
    448-column matmuls per 8-row chunk -- the PE stream is the
    critical path and runs at 1 column/cycle.
  - The 3-tap depthwise conv (t3) moves OFF the PE: ScalarE copies
    each t1 chunk from PSUM into a per-element SBUF tile (bf16, with
    zeroed halo rows), and the DVE accumulates the three h-shifted
    reads with per-partition w3 scalars (fused scalar_tensor_tensor
    MACs), then multiplies against the t4 PSUM bank.
  - Matmuls in bf16 (fp32 matmul lowers to a LOW_HIGH pair at <half
    throughput); accumulation stays fp32 in PSUM.
  - x stays unpadded/contiguous in SBUF; boundary taps are emitted as
    clipped matmuls over row/col sub-ranges (the skipped regions keep
    the value accumulated by the always-full center tap).
  - Output is written bf16 (half the DMA bytes) and upconverted on the
    host; one DMA per batch element, with the last element split so
    the final transfer is a single 8-row chunk (short tail).
  - Small dummy matmuls trip the PE_HAM clock gate (1.2 -> 2.4 GHz)
    while the first DMAs are still streaming in.
"""

import sys

sys.path.insert(0, "/opt/trn_rl_repo")

import ml_dtypes
import numpy as np

import concourse.bacc as bacc
import concourse.bass as bass
import concourse.mybir as mybir
import concourse.tile as tile
from concourse import bass_utils

N, C, H, W, G = 32, 128, 56, 56, 8
NCORES = 8
NPC = N // NCORES  # batch elems per core
CH = 8             # H rows per chunk
NCHUNK = H // CH
H2 = H + 2         # t1 tile rows incl zero halo rows 0 and H+1

F32 = mybir.dt.float32
BF16 = mybir.dt.bfloat16
ALU = mybir.AluOpType

TRACE = False
TRACE_DIR = None
LAST_EXEC_NS = None
LAST_RESULTS = None

_COMPILED = None


def _enable_trace_hook():
    """The agent image's ``antenv`` lacks ``axon_hooks``, so the boot-time
    NTFF hook registration silently degraded. Recreate the module and
    register the same ctypes-based hook; also skip the bucket upload."""
    import sys as _sys
    import types

    if "antenv.axon_hooks" not in _sys.modules:
        mod = types.ModuleType("antenv.axon_hooks")
        mod._hook = None

        def set_axon_ntff_profile_hook(h):
            mod._hook = h

        def get_axon_ntff_profile_hook():
            return mod._hook

        mod.set_axon_ntff_profile_hook = set_axon_ntff_profile_hook
        mod.get_axon_ntff_profile_hook = get_axon_ntff_profile_hook
        _sys.modules["antenv.axon_hooks"] = mod
        import antenv

        antenv.axon_hooks = mod

    from antenv.axon_hooks import get_axon_ntff_profile_hook as _get

    if _get() is None:
        from trn_agent_boot.trn_boot import _ntff_profile_via_ctypes

        hook = _ntff_profile_via_ctypes("/opt/axon/libaxon_pjrt.so")
        if hook is not None:
            _sys.modules["antenv.axon_hooks"].set_axon_ntff_profile_hook(hook)

    bass_utils.upload_artifacts = lambda tmpdir: f"local:{tmpdir}"


def _t1_matmuls(c, pa, xc, wc_t):
    """(lhsT, rhs, out) list accumulating the 5-tap t1 conv for the 8-row
    chunk c, row-clipped at the H borders.  Output row o of the chunk
    reads x row 8c+o+e-2; tap e=2 covers the full chunk for every c and
    is emitted first (start=True)."""
    h0 = c * CH
    mms = []
    for e in (2, 0, 1, 3, 4):
        o_lo = max(0, 2 - e - h0)
        o_hi = min(CH, H + 2 - e - h0)
        if o_lo >= o_hi:
            continue
        r0 = h0 + o_lo + e - 2
        r1 = h0 + o_hi + e - 2
        mms.append((wc_t[:, e, :], xc[:, r0:r1, :], pa[:, o_lo:o_hi, :]))
    return mms


def _t4_matmuls(c, pb, xc, w4_t):
    """t4 chunk: 3 width taps at offsets -2/0/+2, col-clipped at borders."""
    h0 = c * CH
    rows = xc[:, h0 : h0 + CH, :]
    return [
        (w4_t[:, 1, :], rows, pb[:]),                               # delta = 0
        (w4_t[:, 0, :], xc[:, h0 : h0 + CH, 0 : W - 2], pb[:, :, 2:W]),   # -2
        (w4_t[:, 2, :], xc[:, h0 : h0 + CH, 2:W], pb[:, :, 0 : W - 2]),   # +2
    ]


def _build():
    nc = bacc.Bacc(
        "TRN2",
        target_bir_lowering=False,
        debug=False,
        enable_asserts=False,
        num_devices=NCORES,
    )

    x_d = nc.dram_tensor("x_s", (NPC, C, H, W), BF16, kind="ExternalInput").ap()
    wc_d = nc.dram_tensor("wc", (C, 5, C), BF16, kind="ExternalInput").ap()
    w4_d = nc.dram_tensor("w4b", (C, 3, C), BF16, kind="ExternalInput").ap()
    w3_d = nc.dram_tensor("w3c", (C, 3), F32, kind="ExternalInput").ap()
    out_d = nc.dram_tensor("out", (NPC, C, H, W), BF16, kind="ExternalOutput").ap()

    with tile.TileContext(nc) as tc:
        with (
            tc.tile_pool(name="wpool", bufs=1) as wpool,
            tc.tile_pool(name="xpool", bufs=1) as xpool,
            tc.tile_pool(name="t1pool", bufs=2) as t1pool,
            tc.tile_pool(name="tmpool", bufs=2) as tmpool,
            tc.tile_pool(name="opool", bufs=2) as opool,
            tc.tile_pool(name="psA", bufs=3, space="PSUM") as papool,
            tc.tile_pool(name="psB", bufs=3, space="PSUM") as pbpool,
            tc.tile_pool(name="psD", bufs=1, space="PSUM") as pdpool,
        ):
            # Dummy matmuls on a zeroed SBUF strip while the first DMAs
            # stream in: PE_HAM ungates the 2.4 GHz clock only after
            # ~3.4us of sustained activity, so start the clock warming
            # before the real matmuls.  Sized to roughly bridge until the
            # first x piece lands; the garbage results go to a PSUM bank
            # that is never read.  memset on GpSimd (earliest-ready
            # engine) so the PE never waits for it.
            dmy = wpool.tile([C, 256], BF16)
            nc.gpsimd.memset(dmy[:], 0.0)
            dps = pdpool.tile([C, 256], F32)
            for _ in range(6):
                nc.tensor.matmul(
                    dps[:], lhsT=dmy[:, 0:C], rhs=dmy[:], start=True, stop=True
                )

            wc_t = wpool.tile([C, 5, C], BF16)
            w4_t = wpool.tile([C, 3, C], BF16)
            w3_t = wpool.tile([C, 3], F32)

            xcs = []
            for n in range(NPC):
                xc = xpool.tile([C, H, W], BF16, name=f"xc{n}")
                xcs.append(xc)
            # first batch elem lands in two pieces so chunk-0 matmuls can
            # start before the whole tensor arrives
            nc.sync.dma_start(wc_t[:], wc_d[:])
            nc.sync.dma_start(xcs[0][:, 0:12, :], x_d[0, :, 0:12, :])
            nc.sync.dma_start(xcs[0][:, 12:H, :], x_d[0, :, 12:H, :])
            nc.sync.dma_start(w4_t[:], w4_d[:])
            nc.sync.dma_start(w3_t[:], w3_d[:])
            for n in range(1, NPC):
                nc.sync.dma_start(xcs[n][:], x_d[n])

            for n in range(NPC):
                xc = xcs[n]

                # per-elem t1 strip: tile row r holds t1 row r-1; rows 0
                # and H+1 are the depthwise conv's zero padding
                t1s = t1pool.tile([C, H2, W], BF16, name="t1s")
                nc.gpsimd.memset(t1s[:, 0:1, :], 0.0)
                nc.gpsimd.memset(t1s[:, H + 1 : H + 2, :], 0.0)

                ot = opool.tile([C, H, W], BF16, name="ot")
                pbs = [None] * NCHUNK

                def t1_rows(r):
                    return t1s[:, r : r + CH, :]

                def macmul(j):
                    # t3 rows 8j..8j+7 from three shifted t1 reads, then
                    # the elementwise combine against the t4 PSUM bank
                    r0 = j * CH
                    tmpa = tmpool.tile([C, CH, W], BF16, name="tmpa")
                    tmpb = tmpool.tile([C, CH, W], BF16, name="tmpb")
                    t3c = tmpool.tile([C, CH, W], BF16, name="t3c")
                    nc.vector.tensor_scalar_mul(tmpa[:], t1_rows(r0), w3_t[:, 0:1])
                    nc.vector.scalar_tensor_tensor(
                        tmpb[:], t1_rows(r0 + 1), w3_t[:, 1:2], tmpa[:],
                        ALU.mult, ALU.add,
                    )
                    nc.vector.scalar_tensor_tensor(
                        t3c[:], t1_rows(r0 + 2), w3_t[:, 2:3], tmpb[:],
                        ALU.mult, ALU.add,
                    )
                    nc.vector.tensor_mul(
                        ot[:, r0 : r0 + CH, :], t3c[:], pbs[j][:]
                    )

                for c in range(NCHUNK):
                    pa = papool.tile([C, CH, W], F32)
                    mms = _t1_matmuls(c, pa, xc, wc_t)
                    for i, (lhsT, rhs, outap) in enumerate(mms):
                        nc.tensor.matmul(
                            outap,
                            lhsT=lhsT,
                            rhs=rhs,
                            start=(i == 0),
                            stop=(i == len(mms) - 1),
                        )
                    pb = pbpool.tile([C, CH, W], F32)
                    pbs[c] = pb
                    mmsb = _t4_matmuls(c, pb, xc, w4_t)
                    for i, (lhsT, rhs, outap) in enumerate(mmsb):
                        nc.tensor.matmul(
                            outap,
                            lhsT=lhsT,
                            rhs=rhs,
                            start=(i == 0),
                            stop=(i == len(mmsb) - 1),
                        )
                    # t1 chunk -> SBUF strip (bf16) on ScalarE
                    nc.scalar.copy(t1s[:, c * CH + 1 : c * CH + 1 + CH, :], pa[:])
                    if c >= 1:
                        macmul(c - 1)
                macmul(NCHUNK - 1)

                if n < NPC - 1:
                    nc.sync.dma_start(out_d[n], ot[:])
                else:
                    # split the last element's store so the final DMA
                    # after the last vector op is a single short chunk
                    nc.sync.dma_start(
                        out_d[n, :, 0 : H - CH, :], ot[:, 0 : H - CH, :]
                    )
                    nc.sync.dma_start(
                        out_d[n, :, H - CH : H, :], ot[:, H - CH : H, :]
                    )

    nc.compile()
    return nc


def _get_compiled():
    global _COMPILED
    if _COMPILED is None:
        _COMPILED = _build()
    return _COMPILED


def _prep_weights(w1, w3, w4):
    bf = ml_dtypes.bfloat16
    w1c = np.asarray(w1, dtype=np.float32)[:, :, :, 0]  # (co, ci, 5)
    wc = np.ascontiguousarray(w1c.transpose(1, 2, 0))   # (ci, tap, co)
    w3c = np.ascontiguousarray(np.asarray(w3, dtype=np.float32)[:, 0, :, 0])  # (co, 3)
    w4c = np.asarray(w4, dtype=np.float32)[:, :, 0, :]  # (ci, k, g)
    w4b = np.ascontiguousarray(np.tile(w4c, (1, 1, C // G)))  # (ci, k, 128)
    return wc.astype(bf), w3c, w4b.astype(bf)


def kernel(x, w1, w3, w4):
    global LAST_EXEC_NS, LAST_RESULTS
    nc = _get_compiled()
    xb = np.ascontiguousarray(np.asarray(x, dtype=np.float32)).astype(ml_dtypes.bfloat16)
    wc, w3c, w4b = _prep_weights(w1, w3, w4)

    in_maps = [
        {
            "x_s": np.ascontiguousarray(xb[i * NPC : (i + 1) * NPC]),
            "wc": wc,
            "w3c": w3c,
            "w4b": w4b,
        }
        for i in range(NCORES)
    ]
    if TRACE:
        _enable_trace_hook()
    res = bass_utils.run_bass_kernel_spmd(
        nc,
        in_maps,
        core_ids=list(range(NCORES)),
        trace=TRACE,
        tmpdir=TRACE_DIR,
    )
    LAST_EXEC_NS = res.exec_time_ns
    LAST_RESULTS = res
    out = np.concatenate(
        [np.asarray(res.results[i]["out"]) for i in range(NCORES)], axis=0
    ).astype(np.float32)
    return out
